# revision 72
# baseline (speedup 1.0000x reference)
"""AttentionBlock3D on 8 Trainium2 NeuronCores — Gram-matrix restructure, v2.

Math (see reference docstring in kernel_baseline.py): GroupNorm(8 groups)
-> qkv 1x1 conv -> channel attention (contract over tokens N, softmax over
last d=64) -> out proj -> residual.

Sharding: N = T*H*W = 16384 tokens split 8 ways (2048/core); every core
holds all 4 batches of its token slice.  Cross-core state: per-PAIR
GroupNorm stats AllGathers (b0/b1 and b2/b3, 8 KB each) + one logits
AllReduce per batch pair (256 KB each).

Key identity: the channel-attention logits contract over tokens, so
  L = Q K^T = Aq Gx Ak^T + uq bk~^T + bq~ uk^T + N bq~ bk~^T
with Gx = sum_n x x^T (Gram of RAW x — independent of GroupNorm stats!).

v2 schedule changes vs baseline (all driven by the TimelineSim trace):
 - weights stream AFTER x batches 0/1 on the same HWDGE queues, so the
   DMA engines finish x ~12us earlier and the stats collectives fire
   early;
 - stats AllGather is split per batch pair: pair-0 AG completes while
   the PE is still on batch 1's Gram, so phase B for b0/b1 (and the
   first logits AllReduce) starts ~80us earlier; both logits ARs are
   fully hidden behind phase A/B PE work;
 - phase A transposes take f32r inputs (1.5 cy/row instead of 2);
 - the per-batch GN correction vectors are batched: one [4 x 512]
   matmul per (pair, half, co) instead of 16 single-row matmuls/batch;
 - the out-proj bias rank-1 PE matmuls are gone: bias rides the ACT/DVE
   PSUM evacuation;
 - attn@v is folded into the out projection: W2 = Wo A_blockdiag is
   built once per batch (8 small PE matmuls) and the out proj consumes
   v directly, removing 64 [128x512] attn@v matmuls + their evacuations.

All big matmuls run in float32r (free dim 512 -> 1 cycle/row); the rank-1
correction matmuls and transposes are exact fp32/f32r.
"""

import numpy as np

import concourse.bass as bass
import concourse.mybir as mybir
import concourse.tile as tile
from concourse import bass_utils

F32 = mybir.dt.float32
F32R = mybir.dt.float32r
BF16 = mybir.dt.bfloat16
AX = mybir.AxisListType.X
ALU = mybir.AluOpType
ACT = mybir.ActivationFunctionType

N_CORES = 8
B, C, T, H, W = 4, 512, 16, 32, 32
N_TOT = T * H * W            # 16384
NH, D = 8, 64                # heads, head dim
G = 8                        # groupnorm groups
EPS = 1e-5
P = 128
CO = C // P                  # 4 channel chunks
NC = 512                     # phase-C token chunk size


def _round_tf32(a: np.ndarray) -> np.ndarray:
    """Round fp32 to fp32r (keep 10 explicit mantissa bits, RNE)."""
    u = a.astype(np.float32).view(np.uint32).astype(np.uint64)
    u = (u + 0x1000 + ((u >> 13) & 1)) & 0xFFFFE000
    return u.astype(np.uint32).view(np.float32)


KCFG = dict(pin0=1, pin1=2, stats_hi=False, loadt_hi=False)


def build_module(n_loc: int, debug: bool = False):
    NT = n_loc // P              # token blocks per batch (16)
    nchunks = n_loc // NC        # phase-C chunks per batch (4)
    ngr = n_loc // 512           # bn_stats groups per tile
    ntot = n_loc * N_CORES
    m_group = (C // G) * ntot    # elements per (b, group) stat
    scale = float(D) ** -0.5
    NP = B // 2                  # batch pairs

    nc = bass.Bass("TRN2", target_bir_lowering=False, debug=False,
                   num_devices=N_CORES)

    xin = nc.dram_tensor("xin", [B, C, n_loc], F32, kind="ExternalInput").ap()
    wqk_t = nc.dram_tensor("wqk_t", [C, 2 * C], F32, kind="ExternalInput").ap()
    wv_t = nc.dram_tensor("wv_t", [C, C], F32, kind="ExternalInput").ap()
    wo_t = nc.dram_tensor("wo_t", [64, NH, C], F32,
                          kind="ExternalInput").ap()
    qkb = nc.dram_tensor("qkb", [4, 2 * C], F32, kind="ExternalInput").ap()
    vb2 = nc.dram_tensor("vb2", [P, CO], F32, kind="ExternalInput").ap()
    ob2 = nc.dram_tensor("ob2", [P, CO], F32, kind="ExternalInput").ap()
    gnw2 = nc.dram_tensor("gnw2", [P, CO], F32, kind="ExternalInput").ap()
    gnb2 = nc.dram_tensor("gnb2", [P, CO], F32, kind="ExternalInput").ap()
    yout = nc.dram_tensor("yout", [B, C, n_loc], F32, kind="ExternalOutput").ap()
    dbg = {}
    if debug:
        for nm, shp in [("dbg_stats", [P, B, 2, CO]), ("dbg_a", [P, CO, B]),
                        ("dbg_c", [P, CO, B]),
                        ("dbg_logits", [64, B, NH, D]),
                        ("dbg_attn", [64, B, NH, P]),
                        ("dbg_vec", [1, B, 3, 2 * C]),
                        ("dbg_w2", [P, CO, C]),
                        ("dbg_v", [P, CO, NC])]:
            dbg[nm] = nc.dram_tensor(nm, shp, F32, kind="ExternalOutput").ap()

    from concourse.masks import make_identity
    from bass_rust import add_dep_helper as _adh

    with tile.TileContext(nc) as tc:
        with (
            tc.tile_pool(name="persist", bufs=1) as pers,
            tc.tile_pool(name="wvpool", bufs=1) as wvp,
            tc.tile_pool(name="dram", bufs=1, space="DRAM") as dram,
        ):
            # ================= phase A/B =================
            import contextlib
            ab_stack = contextlib.ExitStack()
            abp = ab_stack.enter_context(tc.tile_pool(name="abpers",
                                                      bufs=1))
            xp = ab_stack.enter_context(tc.tile_pool(name="xp", bufs=8))
            xtp = ab_stack.enter_context(tc.tile_pool(name="xtp", bufs=4))
            wqp = ab_stack.enter_context(tc.tile_pool(name="wqp", bufs=1))
            gxsb = ab_stack.enter_context(tc.tile_pool(name="gxsb", bufs=2))
            ysb = ab_stack.enter_context(tc.tile_pool(name="ysb", bufs=1))
            small = ab_stack.enter_context(tc.tile_pool(name="small", bufs=1))
            tps = ab_stack.enter_context(
                tc.tile_pool(name="tps", bufs=2, space="PSUM"))
            gxps = ab_stack.enter_context(
                tc.tile_pool(name="gxps", bufs=1, space="PSUM"))
            ylps = ab_stack.enter_context(
                tc.tile_pool(name="ylps", bufs=2, space="PSUM"))

            # ------------- persistent tiles -------------
            qkb_sb = abp.tile([P, 2 * C], F32)  # rows 64/96 hold qk bias
            vb_sb = pers.tile([P, CO], F32)
            ob_sb = pers.tile([P, CO], F32)
            gnw_sb = pers.tile([P, CO], F32)
            gnb_sb = pers.tile([P, CO], F32)

            ident = abp.tile([P, P], F32)
            make_identity(nc, ident[:])
            # rounded f32r copy for the f32r phase-A transposes (0/1 exact)
            ident_r = abp.tile([P, P], F32R)
            nc.scalar.copy(ident_r[:], ident[:])
            sel_sb = abp.tile([P, 2], F32)
            nc.vector.memset(sel_sb[:], 0.0)
            nc.vector.memset(sel_sb[0:64, 0:1], 1.0)
            nc.vector.memset(sel_sb[64:128, 1:2], 1.0)
            selt_sb = abp.tile([2, P], F32)
            sel_pt = ylps.tile([P, C], F32, tag="yl", name="sel_pt")
            nc.tensor.transpose(sel_pt[0:2, 0:P], sel_sb[:], ident[:])
            nc.vector.tensor_copy(selt_sb[:], sel_pt[0:2, 0:P])

            a_sb = pers.tile([P, CO, B], F32)     # GN scale per (ci,co,b)
            c_sb = pers.tile([P, CO, B], F32)     # GN shift
            logits_sb = abp.tile([P, B, 4, D], mybir.dt.float16)
            # reduced logits, re-laid per HEAD on partitions 0-63 (the
            # AR copy-back DMA scatters odd heads down) so softmax and
            # the W2=Wo*A build run at matmul-legal partition bases
            lg2 = pers.tile([64, B, NH, D], F32)

            # stats layout [P, b, {sum,sumsq}, co] so a batch PAIR is a
            # contiguous slice for the per-pair AllGather
            stats = abp.tile([P, B, 2, CO], F32)   # local sum/sumsq
            stg = abp.tile([P, B, 2, CO], F32)     # global (post-AR)
            bst = abp.tile([P, B, CO, ngr, 6], F32)
            st_in_l = [dram.tile([P, 2, 2, CO], F32, name=f"st_in{p}")
                       for p in range(NP)]
            st_gout_l = [dram.tile([N_CORES, P, 2, 2, CO], F32,
                                   name=f"st_gout{p}") for p in range(NP)]
            # bf16 logits for the AllReduce: halves the payload, the
            # copy-back casts to f32 (gpsimd DMA can cast)
            lg_in_l = [dram.tile([P, 2, 4, D], mybir.dt.float16,
                                 name=f"lg_in{pp}") for pp in range(NP)]
            lg_out_l = [dram.tile([P, 2, 4, D], mybir.dt.float16,
                                  name=f"lg_out{pp}") for pp in range(NP)]

            # rank-1 correction vectors (uqk/8, bqk, N*bqk/8) per batch, at
            # partition 0 (matmul operand bases must be 0/32/64 and equal);
            # the batched vec matmuls land on rows 0-3 and a tiny scatter
            # DMA brings each batch's rows down to partition 0
            cvec = abp.tile([1, B, 3, 2 * C], BF16)

            def uqk8_l(b):
                return cvec[0:1, b, 0]

            def bqk_l(b):
                return cvec[0:1, b, 1]

            def nbq8_l(b):
                return cvec[0:1, b, 2]

            eps_t = abp.tile([2, 1], F32)
            nc.vector.memset(eps_t[:], EPS)

            # batched-vec stationary: columns 0/32/64/96 carry
            # (u_even, u_odd, c_even, c_odd) so the PSUM rows land on
            # valid 32-aligned partition bases; the rest stays zero.
            # f32r to match the wqk moving operand; memset can't target
            # f32r so zeros arrive via an ACT copy from an f32 scratch
            pv_sb = abp.tile([P, CO, P], F32R)
            zf_sb = abp.tile([P, CO, P], F32)
            nc.vector.memset(zf_sb[:], 0.0)
            nc.scalar.copy(pv_sb[:], zf_sb[:])


            # x is loaded as [P, 2, *] co-PAIR half tiles (one DMA per
            # (pair, half) — per-DMA overhead is ~2.2us so bigger is
            # better) on the two HWDGE queues (SP: co 0/1, ACT: co 2/3).
            qpair = [(nc.sync, 0), (nc.sync, 2)]
            NJ = 4                      # transpose quarters per (b, pair)
            gh = ngr // NJ              # bn_stats windows per half per co
            xa_t = {}

            def load_t(b, hi=False):
                # single x pass: [P, 2, 512] co-pair quarters feed BOTH
                # the PE transposes and the DVE bn_stats
                import contextlib as _cl

                def prio():
                    return tc.high_priority() if hi else _cl.nullcontext()
                w = n_loc // NJ
                for jj in range(NJ):
                    for q, (eng, cb) in enumerate(qpair):
                        # f32r tiles: the PE transposes then run at 1.5
                        # cy/row instead of 2 (bits are plain f32; only
                        # the PE treats the label specially)
                        xa = xp.tile([P, 2, w], F32R, tag=f"xt{q}",
                                     name=f"xt{b}_{q}_{jj}")
                        eng.dma_start(
                            xa[:],
                            xin[b, cb * P:(cb + 2) * P, jj * w:(jj + 1) * w]
                            .rearrange("(co ci) n -> ci co n", ci=P)
                            .bitcast(F32R))
                        xa_t[(b, q, jj)] = xa
                        with prio():
                            for cc in range(2):
                                for g in range(gh):
                                    _bn = nc.vector.bn_stats(
                                        bst[:, b, cb + cc, jj * gh + g],
                                        xa[:, cc, g * 512:(g + 1) * 512]
                                        .bitcast(F32))
                                    bn_first.setdefault(b, _bn)
                with prio():
                    stats_aggr(b)

            aggr_last = {}
            bn_first = {}

            def stats_aggr(b):
                for co in range(CO):
                    mvt = small.tile([P, 2], F32, tag="mvt")
                    nc.vector.bn_aggr(mvt[:], bst[:, b, co])
                    nc.vector.tensor_scalar_mul(
                        stats[:, b, 0, co:co + 1], mvt[:, 0:1], float(n_loc))
                    nc.vector.tensor_tensor(
                        stats[:, b, 1, co:co + 1], mvt[:, 0:1],
                        mvt[:, 0:1], ALU.mult)
                    nc.vector.tensor_tensor(
                        stats[:, b, 1, co:co + 1],
                        stats[:, b, 1, co:co + 1], mvt[:, 1:2],
                        ALU.add)
                    aggr_last[b] = nc.vector.tensor_scalar_mul(
                        stats[:, b, 1, co:co + 1],
                        stats[:, b, 1, co:co + 1], float(n_loc))

            # ---- PE: transpose x + accumulate Gram, software-pipelined
            # with lag so the ACT evacuation never stalls the PE ----
            TGX_LAG = 2
            gx_ps = {}
            gsb_l = {}
            gx_last = {}

            pt_first = [None]

            def _emit_t(b, tb):
                tpb = NT // NJ          # tok-blocks per transpose half
                pt = tps.tile([P, C], F32R, tag="t")
                for co in range(CO):
                    q, cc = divmod(co, 2)
                    pt_first[0] = nc.tensor.transpose(
                        pt[:, co * P:(co + 1) * P],
                        xa_t[(b, q, tb // tpb)][:, cc,
                                                (tb % tpb) * P:
                                                (tb % tpb + 1) * P],
                        ident_r[:])
                xt = xtp.tile([P, C], F32R, tag="xt")
                nc.scalar.copy(xt[:], pt[:])
                return xt

            def _emit_gx(b, tb, xt):
                if tb == 0:
                    gx_ps[b] = [gxps.tile([P, C], F32, tag=f"gx{co}",
                                          name=f"gx{b}_{co}")
                                for co in range(CO)]
                for co in range(CO):
                    gx_last[b] = nc.tensor.matmul(
                        gx_ps[b][co][:], xt[:, co * P:(co + 1) * P],
                        xt[:], start=(tb == 0), stop=(tb == NT - 1))
                if tb == NT - 1:
                    # evacuate Gram to SBUF (ACT — DVE runs bn_stats and
                    # must not serialize the Gram pipeline behind them)
                    gsb = gxsb.tile([P, CO, C], F32R, tag="gx",
                                    name=f"gxsb{b}")
                    for co in range(CO):
                        nc.scalar.copy(gsb[:, co], gx_ps[b][co][:])
                    gsb_l[b] = gsb

            def t_gx_run(batches, after=None, rng=None):
                if rng is None:
                    steps = [(b, tb) for b in batches for tb in range(NT)]
                else:
                    steps = [(batches[0], tb) for tb in rng]
                xts = {}
                for i, (b, tb) in enumerate(steps):
                    xts[i] = _emit_t(b, tb)
                    if i == 0 and after is not None:
                        _adh(pt_first[0].ins, after.ins, sync=True,
                             reason="order gram blocks")
                    if i >= TGX_LAG:
                        bb, tt = steps[i - TGX_LAG]
                        _emit_gx(bb, tt, xts.pop(i - TGX_LAG))
                for i in range(len(steps) - TGX_LAG, len(steps)):
                    bb, tt = steps[i]
                    _emit_gx(bb, tt, xts.pop(i))

            # ---- per-pair stats AllGather + local 8-way sum ----
            def stats_ar(p):
                nc.gpsimd.dma_start(st_in_l[p][:], stats[:, 2 * p:2 * p + 2])
                nc.gpsimd.collective_compute(
                    "AllGather", ALU.bypass,
                    replica_groups=[list(range(N_CORES))],
                    ins=[st_in_l[p].opt()], outs=[st_gout_l[p].opt()],
                )

            def stats_red(p):
                stg8 = abp.tile([P, N_CORES, 2 * 2 * CO], F32,
                                 name=f"stg8_{p}")
                nc.gpsimd.dma_start(
                    stg8[:],
                    st_gout_l[p].rearrange("g p b a c -> p g (b a c)"))
                nc.vector.reduce_sum(
                    stg[:, 2 * p:2 * p + 2].rearrange("p b a c -> p (b a c)"),
                    stg8[:].rearrange("p g a -> p a g"), AX)

            def prep_pair(p, ps_pool):
                # group stats for one batch pair: [2(b), 2(stat), CO]
                nf = 2 * 2 * CO
                stats_red(p)
                pt1 = ps_pool.tile([P, C], F32, tag="yl", name=f"prep_ps{p}")
                prep_mm = nc.tensor.matmul(
                    pt1[0:2, 0:nf], sel_sb[:],
                    stg[:, 2 * p:2 * p + 2]
                    .rearrange("p b a c -> p (b a c)"),
                    start=True, stop=True, skip_group_check=True)
                gst = small.tile([2, 2, 2, CO], F32, tag="gst")
                _gstc = nc.vector.tensor_copy(
                    gst[:].rearrange("p b a c -> p (b a c)"), pt1[0:2, 0:nf])
                if 3 in aggr_last:
                    # keep all local bn_stats ahead of the (hi-pri) pair
                    # DVE chain in the DVE order, else AG1 starves
                    _adh(_gstc.ins, aggr_last[3].ins, sync=True,
                         reason="bn before prep DVE")
                mean_t = small.tile([2, 2, CO], F32, tag="mean")
                nc.vector.tensor_scalar_mul(mean_t[:], gst[:, :, 0],
                                            1.0 / m_group)
                ex2_t = small.tile([2, 2, CO], F32, tag="ex2")
                nc.vector.tensor_scalar_mul(ex2_t[:], gst[:, :, 1],
                                            1.0 / m_group)
                var_t = small.tile([2, 2, CO], F32, tag="var")
                nc.vector.tensor_tensor(var_t[:], mean_t[:], mean_t[:],
                                        ALU.mult)
                nc.vector.tensor_tensor(var_t[:], ex2_t[:], var_t[:],
                                        ALU.subtract)
                rstd_t = small.tile([2, 2, CO], F32, tag="rstd")
                nc.scalar.activation(rstd_t[:], var_t[:], ACT.Sqrt,
                                     bias=eps_t[:])
                nc.vector.reciprocal(rstd_t[:], rstd_t[:])
                cg_t = small.tile([2, 2, CO], F32, tag="cg")
                nc.vector.tensor_tensor(cg_t[:], mean_t[:], rstd_t[:],
                                        ALU.mult)
                nc.vector.tensor_scalar_mul(cg_t[:], cg_t[:], -1.0)
                rc2 = small.tile([2, 2, 2, CO], F32, tag="rc2")
                nc.vector.tensor_copy(rc2[:, :, 0], rstd_t[:])
                nc.vector.tensor_copy(rc2[:, :, 1], cg_t[:])
                nc.tensor.matmul(
                    pt1[:, 512 - nf:512], selt_sb[:],
                    rc2[:].rearrange("p b a c -> p (b a c)"),
                    start=True, stop=True, skip_group_check=True)
                bc = small.tile([P, 2, 2, CO], F32, tag="bc")
                nc.vector.tensor_copy(
                    bc[:].rearrange("p b a c -> p (b a c)"),
                    pt1[:, 512 - nf:512])
                # a = rstd*gnw, c = (-mean*rstd)*gnw + gnb, per batch
                for i in range(2):
                    b = 2 * p + i
                    nc.vector.tensor_tensor(a_sb[:, :, b], bc[:, i, 0],
                                            gnw_sb[:], ALU.mult)
                    nc.vector.tensor_tensor(c_sb[:, :, b], bc[:, i, 0+1],
                                            gnw_sb[:], ALU.mult)
                    nc.vector.tensor_tensor(c_sb[:, :, b], c_sb[:, :, b],
                                            gnb_sb[:], ALU.add)
                return prep_mm

            # ---- per-pair batched correction vectors via PE:
            # one [128, 512] matmul per (half, co-accum); meaningful out
            # rows are 0/32 (u even/odd) and 64/96 (Wc even/odd)
            def vec_pair(p, ps_pool):
                for i in range(2):
                    b = 2 * p + i
                    nc.vector.tensor_tensor(
                        pv_sb[:, :, 32 * i:32 * i + 1],
                        a_sb[:, :, b:b + 1],
                        stg[:, b, 0:1].rearrange("p a c -> p c a"),
                        ALU.mult)
                    nc.vector.tensor_copy(
                        pv_sb[:, :, 64 + 32 * i:65 + 32 * i],
                        c_sb[:, :, b:b + 1])
                vtmp = small.tile([P, 2, 2 * C], BF16, tag="vtmp",
                                  name=f"vtmp{p}")
                for half in range(2):
                    vp = ps_pool.tile([P, C], F32, tag="yl",
                                      name=f"vec{p}_{half}")
                    sl = slice(half * 512, half * 512 + 512)
                    for co in range(CO):
                        nc.tensor.matmul(
                            vp[:], pv_sb[:, co], wqk_sb[:, co, sl],
                            start=(co == 0), stop=(co == CO - 1),
                            skip_group_check=True)
                    # uqk8 = u/8 on rows 0/32; bqk = Wc + qkv bias on rows
                    # 64/96 slot 0; nbq8 = N*bqk/8 on rows 64/96 slot 1
                    for i in range(2):
                        r_u, r_c = 32 * i, 64 + 32 * i
                        nc.vector.tensor_scalar_mul(
                            vtmp[r_u:r_u + 1, 0, sl], vp[r_u:r_u + 1, :],
                            1.0 / N_CORES)
                        nc.vector.tensor_tensor(
                            vtmp[r_c:r_c + 1, 0, sl], vp[r_c:r_c + 1, :],
                            qkb_sb[r_c:r_c + 1, sl], ALU.add)
                        nc.vector.tensor_scalar_mul(
                            vtmp[r_c:r_c + 1, 1, sl],
                            vtmp[r_c:r_c + 1, 0, sl],
                            float(ntot) / N_CORES)
                # scatter down to partition 0 per batch (HWDGE queues —
                # the Pool FIFO must stay clear for the collectives; high
                # priority so phase-C x prefetches don't queue ahead)
                with tc.high_priority():
                    for i in range(2):
                        b = 2 * p + i
                        nc.sync.dma_start(cvec[0:1, b, 0],
                                          vtmp[32 * i:32 * i + 1, 0])
                        nc.sync.dma_start(cvec[0:1, b, 1:3],
                                          vtmp[64 + 32 * i:65 + 32 * i,
                                               0:2])


            # ---- per-batch: Y = (a.Gx) Wk^T, Y' = a.Y, L = Wq^T Y' ----
            def y_l(b, gsb):
                for co in range(CO):
                    nc.vector.tensor_scalar_mul(gsb[:, co], gsb[:, co],
                                                a_sb[:, co, b:b + 1])
                y_sb = ysb.tile([P, CO, C], F32R, tag="y", name=f"ysb{b}")
                for c1 in range(CO):
                    yp = ylps.tile([P, C], F32, tag="yl", name=f"y{b}_{c1}")
                    for c2 in range(CO):
                        nc.tensor.matmul(
                            yp[:], gsb[:, c2, c1 * P:(c1 + 1) * P],
                            wqk_sb[:, c2, C:2 * C],
                            start=(c2 == 0), stop=(c2 == CO - 1))
                    if c1 % 2 == 0:
                        nc.scalar.activation(y_sb[:, c1], yp[:],
                                             ACT.Identity,
                                             scale=a_sb[:, c1, b:b + 1])
                    else:
                        nc.vector.tensor_scalar_mul(y_sb[:, c1], yp[:],
                                                    a_sb[:, c1, b:b + 1])
                last_mm = None
                for dc in range(CO):
                    lp = ylps.tile([P, C], F32, tag="yl", name=f"l{b}_{dc}")
                    for c1 in range(CO):
                        nc.tensor.matmul(
                            lp[:], wqk_sb[:, c1, dc * P:(dc + 1) * P],
                            y_sb[:, c1], start=(c1 == 0), stop=False,
                            skip_group_check=True)
                    # rank-1 corrections on the two diagonal head blocks
                    for par in range(2):
                        hh = 2 * dc + par
                        rows = slice(par * 64, par * 64 + 64)
                        cols = slice(hh * 64, hh * 64 + 64)
                        tp = (0, 64) if par else None
                        ksl = slice(C + hh * 64, C + hh * 64 + 64)
                        qsl = slice(hh * 64, hh * 64 + 64)
                        nc.tensor.matmul(
                            lp[rows, cols], uqk8_l(b)[:, qsl],
                            bqk_l(b)[:, ksl], start=False, stop=False,
                            tile_position=tp, skip_group_check=True)
                        nc.tensor.matmul(
                            lp[rows, cols], bqk_l(b)[:, qsl],
                            uqk8_l(b)[:, ksl], start=False, stop=False,
                            tile_position=tp, skip_group_check=True)
                        last_mm = nc.tensor.matmul(
                            lp[rows, cols], nbq8_l(b)[:, qsl],
                            bqk_l(b)[:, ksl], start=False, stop=(par == 1),
                            tile_position=tp, skip_group_check=True)
                    # extract diagonal head blocks (alternate DVE/ACT to
                    # halve the serialized per-op SEQ latency chain)
                    for par in range(2):
                        hh = 2 * dc + par
                        rows = slice(par * 64, par * 64 + 64)
                        if par == 0:
                            nc.scalar.copy(
                                logits_sb[rows, b, dc, :],
                                lp[rows, hh * 64:hh * 64 + 64])
                        else:
                            nc.vector.tensor_copy(
                                logits_sb[rows, b, dc, :],
                                lp[rows, hh * 64:hh * 64 + 64])
                return last_mm

            def lg_ar_in(p):
                # paired logits AllReduce: batches 2p, 2p+1 in one 256KB op
                nc.gpsimd.dma_start(lg_in_l[p][:],
                                    logits_sb[:, 2 * p:2 * p + 2])
                nc.gpsimd.collective_compute(
                    "AllReduce", ALU.add,
                    replica_groups=[list(range(N_CORES))],
                    ins=[lg_in_l[p].opt()], outs=[lg_out_l[p].opt()],
                )

            def lg_ar_out(p):
                # emitted after BOTH collectives so the copy-back of pair 0
                # does not block pair 1's input DMA in the Pool FIFO.
                # Scatters even/odd head rows to [64, b, head, d] layout.
                nc.gpsimd.dma_start(lg2[0:64, 2 * p:2 * p + 2, 0:NH:2],
                                    lg_out_l[p][0:64])
                nc.gpsimd.dma_start(lg2[0:64, 2 * p:2 * p + 2, 1:NH:2],
                                    lg_out_l[p][64:128])

            # ============ emit phase A/B in PE-schedule order ============
            # DMA order on the HWDGE queues: consts, x(b0), x(b1), wqk,
            # x(b2), x(b3), wv, wo — weights ride behind the x pairs they
            # don't delay, and each pair's stats AG fires right after its
            # bn_stats complete.
            nc.gpsimd.dma_start(qkb_sb[64:65], qkb[2:3])
            nc.gpsimd.dma_start(qkb_sb[96:97], qkb[3:4])
            nc.gpsimd.dma_start(vb_sb[:], vb2[:])
            nc.gpsimd.dma_start(ob_sb[:], ob2[:])
            nc.gpsimd.dma_start(gnw_sb[:], gnw2[:])
            nc.gpsimd.dma_start(gnb_sb[:], gnb2[:])
            load_t(0, hi=KCFG["loadt_hi"])
            load_t(1, hi=KCFG["loadt_hi"])
            if KCFG["stats_hi"]:
                with tc.high_priority():
                    stats_ar(0)
            else:
                stats_ar(0)
            wqk_sb = wqp.tile([P, CO, 2 * C], F32R)
            for half in range(2):
                nc.sync.dma_start(
                    wqk_sb[:, :, half * C:(half + 1) * C],
                    wqk_t[:, half * C:(half + 1) * C]
                    .rearrange("(co ci) o -> ci co o", ci=P)
                    .bitcast(F32R))
            load_t(2)
            load_t(3)
            with tc.high_priority():
                stats_ar(1)
            wv_sb = wvp.tile([P, CO, C], F32R)
            wo2_sb = wvp.tile([64, NH, C], F32R)
            nc.sync.dma_start(
                wv_sb[:],
                wv_t.rearrange("(co ci) o -> ci co o", ci=P).bitcast(F32R))
            nc.sync.dma_start(wo2_sb[:], wo_t.bitcast(F32R))

            t_gx_run([0])
            t_gx_run([1])
            # pair-0 stats must not be threshold-coupled to b2/b3 bn_stats
            _adh(bn_first[2].ins, aggr_last[1].ins, sync=True,
                 reason="pair0 stats before b2 bn")
            # phase-B pair 0 runs right after A1 while AG0 is in flight
            with tc.high_priority():
                m1 = prep_pair(0, ylps)
                _adh(m1.ins, gx_last[1].ins, sync=True, reason="prep0 pin")
                vec_pair(0, ylps)
                y_l(0, gsb_l.pop(0))
                y_l(1, gsb_l.pop(1))
                lg_ar_in(0)
            t_gx_run([2])
            t_gx_run([3])
            with tc.high_priority():
                prep_pair(1, ylps)
                vec_pair(1, ylps)
                y_l(2, gsb_l.pop(2))
                y_l(3, gsb_l.pop(3))
                lg_ar_out(0)  # pair-0 copy-back first: it unblocks
                lg_ar_in(1)   # softmax(0)/W2(0) ~5us before AR23's input
                lg_ar_out(1)  # is ready anyway
            _ = gh  # silence lint; gh used by load_t

            if debug:
                nc.sync.dma_start(dbg["dbg_a"][:], a_sb[:])
                nc.sync.dma_start(dbg["dbg_c"][:], c_sb[:])
                nc.sync.dma_start(dbg["dbg_stats"][:], stg[:])
                nc.sync.dma_start(dbg["dbg_logits"][:], lg2[:])
                nc.sync.dma_start(dbg["dbg_vec"][:], cvec[:])
            ab_stack.close()

            # ================= phase C =================
            with (
                tc.tile_pool(name="cpers", bufs=1) as cpers,
                tc.tile_pool(name="xc", bufs=8) as xc,
                tc.tile_pool(name="hp", bufs=2) as hp,
                tc.tile_pool(name="vp", bufs=5) as vp,
                tc.tile_pool(name="w2p", bufs=2) as w2p,
                tc.tile_pool(name="yp", bufs=2) as yp,
                tc.tile_pool(name="smp", bufs=4) as smp,
                tc.tile_pool(name="cps", bufs=6, space="PSUM") as cps,
                tc.tile_pool(name="taps", bufs=2, space="PSUM") as taps,
            ):
                # softmaxed attention per head at base 0, f32r, stored
                # zero-padded to 128 columns: even heads at cols 0-63,
                # odd heads at cols 64-127, the other half zero.  The W2
                # build then needs no PE tiling: both heads of a chunk
                # accumulate into one [128, 512] PSUM group.
                attn3 = cpers.tile([64, B, NH, P], F32R)
                zq = cpers.tile([64, D], F32)
                nc.vector.memset(zq[:], 0.0)
                for b in range(B):
                    for h in range(NH):
                        opp = slice(0, 64) if h % 2 else slice(64, 128)
                        eng = nc.scalar if h % 2 else nc.vector
                        if h % 2:
                            nc.scalar.copy(attn3[0:64, b, h, opp], zq[:])
                        else:
                            nc.vector.tensor_copy(attn3[0:64, b, h, opp],
                                                  zq[:])

                def softmax_b(b):
                    for h in range(NH):
                        blk = lg2[0:64, b, h]
                        mx = smp.tile([64, 1], F32, tag="mx")
                        nc.vector.reduce_max(mx[:], blk, AX)
                        nbias = smp.tile([64, 1], F32, tag="nb")
                        nc.vector.tensor_scalar_mul(nbias[:], mx[:], -scale)
                        ex = attn3[0:64, b, h, (h % 2) * 64:(h % 2) * 64 + 64]
                        nc.scalar.activation(ex, blk, ACT.Exp, bias=nbias[:],
                                             scale=scale)
                        sm = smp.tile([64, 1], F32, tag="sm")
                        nc.vector.reduce_sum(sm[:], ex, AX)
                        nc.vector.reciprocal(sm[:], sm[:])
                        nc.vector.tensor_scalar_mul(ex, ex, sm[:])

                def build_w2(b):
                    # W2 = (Wo A_blockdiag)^T in wo_sb layout: per co the
                    # two heads' [64 x 512] products land on out rows
                    # 0-63 / 64-127 via the zero-padded 128-wide lhsT.
                    w2 = w2p.tile([P, CO, C], F32R, tag="w2",
                                  name=f"w2_{b}")
                    for co in range(CO):
                        tp_t = taps.tile([P, C], F32, tag="taps")
                        for par in range(2):
                            h = 2 * co + par
                            nc.tensor.matmul(
                                tp_t[:], attn3[0:64, b, h],
                                wo2_sb[0:64, h, :],
                                start=(par == 0), stop=(par == 1))
                        if co % 2 == 0:
                            nc.scalar.copy(w2[:, co], tp_t[:])
                        else:
                            nc.vector.tensor_copy(w2[:, co], tp_t[:])
                    if debug and b == 0:
                        nc.gpsimd.dma_start(dbg["dbg_w2"][:], w2[:])
                    return w2

                def emit_v(b, j):
                    xv = xin[b].rearrange("(co ci) n -> ci co n", ci=P)
                    xa = xc.tile([P, CO, NC], F32, tag="x")
                    nc.sync.dma_start(xa[:], xv[:, :, j * NC:(j + 1) * NC])
                    h = hp.tile([P, CO, NC], F32R, tag="h")
                    for co in range(CO):
                        nc.vector.tensor_scalar(
                            h[:, co], xa[:, co],
                            a_sb[:, co, b:b + 1], c_sb[:, co, b:b + 1],
                            ALU.mult, ALU.add)
                    v = vp.tile([P, CO, NC], F32R, tag="v")
                    for ot in range(CO):
                        ps_v = cps.tile([P, NC], F32, tag="c")
                        for co in range(CO):
                            nc.tensor.matmul(
                                ps_v[:], wv_sb[:, co, ot * P:(ot + 1) * P],
                                h[:, co], start=(co == 0), stop=(co == CO - 1))
                        if ot % 2 == 0:
                            nc.scalar.activation(v[:, ot], ps_v[:],
                                                 ACT.Identity,
                                                 bias=vb_sb[:, ot:ot + 1])
                        else:
                            nc.vector.tensor_scalar_add(
                                v[:, ot], ps_v[:], vb_sb[:, ot:ot + 1])
                    if debug and b == 0 and j == 0:
                        nc.gpsimd.dma_start(dbg["dbg_v"][:], v[:])
                    return xa, v

                def finish(b, j, xa, v, w2):
                    yv = yout[b].rearrange("(co ci) n -> ci co n", ci=P)
                    y_sb = yp.tile([P, CO, NC], F32, tag="y")
                    for ot in range(CO):
                        ps_o = cps.tile([P, NC], F32, tag="c")
                        for co in range(CO):
                            nc.tensor.matmul(
                                ps_o[:], w2[:, co, ot * P:(ot + 1) * P],
                                v[:, co], start=(co == 0), stop=(co == CO - 1))
                        # out bias on the ACT/DVE evacuation, residual on
                        # the other engine
                        if ot % 2 == 0:
                            nc.scalar.activation(y_sb[:, ot], ps_o[:],
                                                 ACT.Identity,
                                                 bias=ob_sb[:, ot:ot + 1])
                            nc.vector.tensor_tensor(
                                y_sb[:, ot], y_sb[:, ot], xa[:, ot], ALU.add)
                        else:
                            nc.vector.tensor_scalar_add(
                                y_sb[:, ot], ps_o[:], ob_sb[:, ot:ot + 1])
                            nc.vector.tensor_tensor(
                                y_sb[:, ot], y_sb[:, ot], xa[:, ot], ALU.add)
                        if b == B - 1:
                            # last batch: store per-ot on two queues so
                            # the final DMAs drain behind each out-proj
                            eng = nc.gpsimd if ot % 2 == 0 else nc.scalar
                            eng.dma_start(
                                yv[:, ot, j * NC:(j + 1) * NC], y_sb[:, ot])
                    if b != B - 1:
                        nc.gpsimd.dma_start(
                            yv[:, :, j * NC:(j + 1) * NC], y_sb[:])

                softmax_b(0)
                for b in range(B):
                    pend = []
                    for j in range(nchunks):
                        pend.append((j, *emit_v(b, j)))
                    w2 = build_w2(b)
                    if b + 1 < B:
                        # prefetch next batch's softmax so its W2
                        # matmuls don't stall the PE
                        softmax_b(b + 1)
                    for j, xa, v in pend:
                        finish(b, j, xa, v, w2)
                if debug:
                    nc.sync.dma_start(dbg["dbg_attn"][:], attn3[:])

    return nc


_WAITSPLIT_COUNTER = [0]


def _split_waits(nc, limit: int = 1):
    """Walrus in this container rejects instructions with more than one sync
    wait; split extras onto injected NoOps on the same engine."""
    n_split = 0
    for fn in nc.m.functions:
        for bb in fn.blocks:
            insts = list(bb.instructions)
            out = []
            changed = False
            for inst in insts:
                si = inst.sync_info
                waits = list(si.on_wait) if si is not None and si.on_wait \
                    else []
                if len(waits) > limit:
                    keep = waits[-limit:]
                    extra = waits[:-limit]
                    for i in range(0, len(extra), limit):
                        chunk = extra[i:i + limit]
                        _WAITSPLIT_COUNTER[0] += 1
                        nop = mybir.InstNoOp(
                            name=f"waitsplit-{_WAITSPLIT_COUNTER[0]}",
                            ins=[], outs=[])
                        nop.engine = inst.engine
                        nop.sync_info = mybir.SyncInfo(
                            on_wait=chunk, on_update=[])
                        out.append(nop)
                    si.on_wait = keep
                    n_split += 1
                    changed = True
                out.append(inst)
            if changed:
                bb.instructions = out
    return n_split


_CACHE = {}


def _get_module(n_loc, split=True, debug=False):
    key = (n_loc, split, debug)
    if key not in _CACHE:
        nc = build_module(n_loc, debug=debug)
        if split:
            _split_waits(nc, limit=1)
        _CACHE[key] = nc
    return _CACHE[key]


def make_in_maps(inputs, n_loc=None):
    x = np.ascontiguousarray(np.asarray(inputs["x"], dtype=np.float32))
    qkv_w = np.asarray(inputs["qkv_w"], dtype=np.float32)
    qkv_b = np.asarray(inputs["qkv_b"], dtype=np.float32)
    out_w = np.asarray(inputs["out_w"], dtype=np.float32)
    out_b = np.asarray(inputs["out_b"], dtype=np.float32)
    gn_w = np.asarray(inputs["gn_weight"], dtype=np.float32)
    gn_b = np.asarray(inputs["gn_bias"], dtype=np.float32)

    n_tot = int(np.prod(x.shape[2:]))
    if n_loc is None:
        n_loc = n_tot // N_CORES
    xf = x.reshape(B, C, n_tot)

    wqk_t = np.ascontiguousarray(_round_tf32(qkv_w[0:2 * C].T))
    wv_t = np.ascontiguousarray(_round_tf32(qkv_w[2 * C:3 * C].T))
    # Wo^T re-laid per head: [d(64), head(8), out(512)]
    wo_t = np.ascontiguousarray(
        _round_tf32(out_w.T).reshape(NH, 64, C).transpose(1, 0, 2))
    qkb = np.ascontiguousarray(
        np.tile(qkv_b[0:2 * C].reshape(1, 2 * C), (4, 1)))
    vb2 = np.ascontiguousarray(qkv_b[2 * C:3 * C].reshape(CO, P).T)
    ob2 = np.ascontiguousarray(out_b.reshape(CO, P).T)
    gnw2 = np.ascontiguousarray(gn_w.reshape(CO, P).T)
    gnb2 = np.ascontiguousarray(gn_b.reshape(CO, P).T)

    shared = dict(wqk_t=wqk_t, wv_t=wv_t, wo_t=wo_t, qkb=qkb, vb2=vb2,
                  ob2=ob2, gnw2=gnw2, gnb2=gnb2)
    in_maps = []
    for c in range(N_CORES):
        sl = np.ascontiguousarray(xf[:, :, c * n_loc:(c + 1) * n_loc])
        in_maps.append({"xin": sl, **shared})
    return in_maps


def run(inputs, n_loc=None, debug=False, **kw):
    x = np.asarray(inputs["x"])
    n_tot = int(np.prod(x.shape[2:]))
    if n_loc is None:
        n_loc = n_tot // N_CORES
    nc = _get_module(n_loc, debug=debug)
    in_maps = make_in_maps(inputs, n_loc)
    res = bass_utils.run_bass_kernel_spmd(
        nc, in_maps, core_ids=list(range(N_CORES)), **kw)
    y = np.concatenate([res.results[c]["yout"] for c in range(N_CORES)],
                       axis=2)
    return y, res


def kernel(**inputs) -> np.ndarray:
    x = np.asarray(inputs["x"])
    y, _ = run(inputs)
    return y.reshape(x.shape).astype(np.asarray(x).dtype)
